# revision 7
# baseline (speedup 1.0000x reference)
"""Masked multi-head SDP attention on 8 NeuronCores (head-parallel).

B=4, S=2048, D=1024, H=16, DK=64. Each core owns 2 heads: computes
Q/K/V projections for its heads (x replicated, DMA'd directly in
[d, s] transposed layout), causal flash-style attention with scores
kept transposed ([t, sq]) so the attn@V matmul needs no transposes,
softmax denominators fused into the V matmul via an appended ones
column, per-head normalization, and a partial output projection
through its 128 rows of W_O. Host sums the 8 partials and adds b_o.

Matmuls run as float32r (full PE rate at free-dim >= 256, ~1e-3 max
relative error vs fp32 -- measured on HW).
"""

import sys

sys.path.insert(0, "/opt/trn_rl_repo")

import numpy as np

import concourse.bass as bass
import concourse.mybir as mybir
from concourse import bacc
from concourse.masks import make_identity
from concourse.tile import TileContext
from concourse.bass_utils import run_bass_kernel_spmd

B, S, D, H = 4, 2048, 1024, 16
DK = D // H  # 64
NCORES = 8
HPC = H // NCORES  # 2 heads per core
KH = HPC * DK  # 128 = stacked head dim per core
NT = S // 128  # 16 t-tiles per batch
NI = S // 512  # 4 sq-blocks per batch
DC = D // 128  # 8 d-chunks

F32 = mybir.dt.float32
F32R = mybir.dt.float32r


def build_nc():
    nc = bacc.Bacc("TRN2", target_bir_lowering=False, debug=False,
                   num_devices=NCORES)
    x = nc.dram_tensor("x", [B, S, D], F32R, kind="ExternalInput").ap()
    wq = nc.dram_tensor("wq", [DC, 128, KH], F32R, kind="ExternalInput").ap()
    wk = nc.dram_tensor("wk", [DC, 128, KH], F32R, kind="ExternalInput").ap()
    wv = nc.dram_tensor("wv", [DC, 128, KH], F32R, kind="ExternalInput").ap()
    bq = nc.dram_tensor("bq", [KH, 1], F32, kind="ExternalInput").ap()
    bk = nc.dram_tensor("bk", [KH, 1], F32, kind="ExternalInput").ap()
    bv = nc.dram_tensor("bv", [KH, 1], F32, kind="ExternalInput").ap()
    wo = nc.dram_tensor("wo", [KH, D], F32R, kind="ExternalInput").ap()
    tri = nc.dram_tensor("tri", [128, 128], F32R, kind="ExternalInput").ap()
    out = nc.dram_tensor("out", [B, S, D], F32, kind="ExternalOutput").ap()

    with TileContext(nc) as tc:
        with (
            tc.tile_pool(name="const", bufs=1) as cpool,
            tc.tile_pool(name="wts", bufs=1) as wpool,
            tc.tile_pool(name="xt", bufs=2) as xpool,
            tc.tile_pool(name="seq", bufs=1) as qpool,
            tc.tile_pool(name="vn", bufs=2) as vpool,
            tc.tile_pool(name="attn", bufs=4) as apool,
            tc.tile_pool(name="fin", bufs=2) as fpool,
            tc.tile_pool(name="pacc", bufs=2, space="PSUM") as ps_acc,
            tc.tile_pool(name="psc", bufs=3, space="PSUM") as ps_sc,
            tc.tile_pool(name="pv", bufs=2, space="PSUM") as ps_v,
        ):
            # persistent constants
            ident = cpool.tile([128, 128], F32, tag="ident")
            make_identity(nc, ident[:])
            ones_r = cpool.tile([128, 64], F32R, tag="ones")
            nc.gpsimd.memset(ones_r[:].bitcast(F32), 1.0)
            tri_sb = cpool.tile([128, 128], F32R, tag="tri")
            nc.sync.dma_start(out=tri_sb[:], in_=tri)
            w_sb = {}
            for nm, src in (("q", wq), ("k", wk), ("v", wv)):
                t = cpool.tile([128, DC * KH], F32R, tag="w" + nm)
                nc.sync.dma_start(
                    out=t[:].rearrange("p (c k) -> p c k", c=DC),
                    in_=src.rearrange("c p k -> p c k"))
                w_sb[nm] = t
            b_sb = {}
            for nm, src in (("q", bq), ("k", bk), ("v", bv)):
                t = cpool.tile([KH, 1], F32, tag="b" + nm)
                nc.sync.dma_start(out=t[:], in_=src)
                b_sb[nm] = t
            wo_sb = cpool.tile([KH, D], F32R, tag="wo")
            nc.sync.dma_start(out=wo_sb[:], in_=wo)

            for b in range(B):
                # ---------------- projections ----------------
                qt_sb = qpool.tile([128, S], F32R, tag="qt")
                kt_sb = qpool.tile([128, S], F32R, tag="kt")
                vn = [vpool.tile([128, 2 * DK + 2], F32R, tag=f"vn{j}",
                                 name=f"vn_{b}_{j}") for j in range(NT)]
                for st in range(NI):
                    xts = []
                    for dc in range(DC):
                        xt = xpool.tile([128, 512], F32R, tag=f"xt{dc}")
                        nc.sync.dma_start(
                            out=xt[:],
                            in_=x[b, st * 512:(st + 1) * 512,
                                  dc * 128:(dc + 1) * 128]
                            .rearrange("s d -> d s"))
                        xts.append(xt)
                    for nm, dst in (("q", qt_sb), ("k", kt_sb), ("v", None)):
                        acc = ps_acc.tile([128, 512], F32, tag="acc")
                        for dc in range(DC):
                            nc.tensor.matmul(
                                acc[:], w_sb[nm][:, dc * KH:(dc + 1) * KH],
                                xts[dc][:], start=(dc == 0), stop=(dc == DC - 1))
                        if nm != "v":
                            # psum -> sbuf with per-partition bias add
                            nc.vector.tensor_scalar_add(
                                dst[:, st * 512:(st + 1) * 512], acc[:],
                                b_sb[nm][:])
                        else:
                            vtt = fpool.tile([128, 512], F32, tag="vtt")
                            nc.vector.tensor_scalar_add(vtt[:], acc[:],
                                                        b_sb["v"][:])
                            for q in range(4):  # transpose to V natural
                                j = st * 4 + q
                                tp = ps_sc.tile([128, 128], F32, tag="sc")
                                nc.tensor.transpose(
                                    tp[:], vtt[:, q * 128:(q + 1) * 128],
                                    ident[:])
                                nc.vector.tensor_copy(vn[j][:, 0:DK],
                                                      tp[:, 0:DK])
                                nc.vector.tensor_copy(
                                    vn[j][:, DK + 1:2 * DK + 1],
                                    tp[:, DK:2 * DK])
                                nc.gpsimd.memset(vn[j][:, DK:DK + 1].bitcast(F32), 1.0)
                                nc.gpsimd.memset(
                                    vn[j][:, 2 * DK + 1:2 * DK + 2]
                                    .bitcast(F32), 1.0)

                # ---------------- attention ----------------
                catt = fpool.tile([128, S], F32R, tag="catt")
                for i in range(NI):
                    vps = [ps_v.tile([65, 512], F32, tag="vv",
                                     name=f"vp_{b}_{i}_{h}") for h in range(2)]
                    for j in range(4 * i + 4):
                        off = 128 * (j - 4 * i) if j >= 4 * i else 0
                        w = 512 - off
                        sq0 = i * 512 + off
                        for h in range(2):
                            kslc = slice(h * DK, (h + 1) * DK)
                            sp = ps_sc.tile([128, 512], F32, tag="sc")
                            nc.tensor.matmul(
                                sp[:, off:], kt_sb[kslc, j * 128:(j + 1) * 128],
                                qt_sb[kslc, sq0:i * 512 + 512],
                                start=True, stop=True)
                            at = apool.tile([128, 512], F32R, tag=f"at{h}")
                            nc.scalar.activation(
                                at[:, 0:w], sp[:, off:],
                                mybir.ActivationFunctionType.Exp,
                                scale=float(1.0 / np.sqrt(DK)))
                            if j >= 4 * i:
                                nc.vector.tensor_mul(at[:, 0:128],
                                                     at[:, 0:128], tri_sb[:])
                            nc.tensor.matmul(
                                vps[h][:, off:],
                                vn[j][:, h * (DK + 1):(h + 1) * (DK + 1)],
                                at[:, 0:w],
                                start=(j == 0), stop=(j == 4 * i + 3))
                    # normalize: recip of denom row, broadcast via PE, multiply
                    dn = fpool.tile([65, 1024], F32R, tag="dn")
                    for h in range(2):
                        nc.vector.tensor_copy(dn[64:65, h * 512:(h + 1) * 512],
                                              vps[h][64:65, :])
                    with nc.allow_low_precision(reason="f32r == f32 bytes"):
                        nc.vector.reciprocal(dn[64:65, :], dn[64:65, :])
                    nat1 = fpool.tile([64, 512], F32R, tag="nat1")
                    for h in range(2):
                        bc = ps_sc.tile([64, 512], F32, tag="sc")
                        nc.tensor.matmul(bc[:], ones_r[64:65, :],
                                         dn[64:65, h * 512:(h + 1) * 512],
                                         start=True, stop=True)
                        bcs = fpool.tile([64, 512], F32R, tag="bcs",
                                         name=f"bcs_{b}_{i}_{h}")
                        nc.vector.tensor_copy(bcs[:], bc[:])
                        dst = (catt[0:64, i * 512:(i + 1) * 512] if h == 0
                               else nat1[:])
                        nc.vector.tensor_mul(dst, vps[h][0:64, :], bcs[:])
                    # cross-partition hop: head1 rows into catt[64:128]
                    nc.sync.dma_start(
                        out=catt[64:128, i * 512:(i + 1) * 512], in_=nat1[:])

                # ---------------- output projection ----------------
                for st in range(NT):
                    ob = fpool.tile([128, D], F32, tag="ob")
                    for half in range(2):
                        pw = ps_sc.tile([128, 512], F32, tag="sc")
                        nc.tensor.matmul(
                            pw[:], catt[:, st * 128:(st + 1) * 128],
                            wo_sb[:, half * 512:(half + 1) * 512],
                            start=True, stop=True)
                        nc.scalar.copy(ob[:, half * 512:(half + 1) * 512],
                                       pw[:])
                    nc.sync.dma_start(
                        out=out[b, st * 128:(st + 1) * 128, :], in_=ob[:])
    nc.finalize()
    return nc


_NC_CACHE = {}


def _get_nc():
    if "nc" not in _NC_CACHE:
        _NC_CACHE["nc"] = build_nc()
    return _NC_CACHE["nc"]


def kernel(x, Wq, bq, Wk, bk, Wv, bv, Wo, bo):
    x = np.ascontiguousarray(np.asarray(x, dtype=np.float32))
    tri = np.triu(np.ones((128, 128), dtype=np.float32))
    in_maps = []
    for c in range(NCORES):
        h0, h1 = 2 * c, 2 * c + 1
        m = {
            "x": x,
            "tri": tri,
            "wo": np.ascontiguousarray(Wo[c * KH:(c + 1) * KH]).astype(
                np.float32),
        }
        for nm, W, bb in (("q", Wq, bq), ("k", Wk, bk), ("v", Wv, bv)):
            Wc = np.concatenate([W[h0], W[h1]], axis=1).astype(np.float32)
            m["w" + nm] = np.ascontiguousarray(Wc.reshape(DC, 128, KH))
            m["b" + nm] = np.concatenate([bb[h0], bb[h1]]).astype(
                np.float32).reshape(KH, 1)
        in_maps.append(m)
    nc = _get_nc()
    res = run_bass_kernel_spmd(nc, in_maps, list(range(NCORES)))
    acc = np.zeros((B, S, D), dtype=np.float32)
    for c in range(NCORES):
        acc += res.results[c]["out"]
    return acc + np.asarray(bo, dtype=np.float32)[None, None, :]


# revision 16
# speedup vs baseline: 7.8571x; 7.8571x over previous
"""Masked multi-head SDP attention on 8 NeuronCores (head-parallel).

B=4, S=2048, D=1024, H=16, DK=64. Each core owns 2 heads: computes
Q/K/V projections for its heads (x replicated, DMA'd directly in
[d, s] transposed layout), causal flash-style attention with scores
kept transposed ([t, sq]) so the attn@V matmul needs no transposes,
softmax denominators fused into the V matmul via an appended ones
column, per-head normalization, and a partial output projection
through its 128 rows of W_O. Host sums the 8 partials and adds b_o.

Matmuls run as float32r (full PE rate at free-dim >= 256, ~1e-3 max
relative error vs fp32 -- measured on HW).
"""

import sys

sys.path.insert(0, "/opt/trn_rl_repo")

import numpy as np

import concourse.bass as bass
import concourse.mybir as mybir
from concourse import bacc
from concourse.masks import make_identity
from concourse.tile import TileContext
from concourse.bass_utils import run_bass_kernel_spmd

B, S, D, H = 4, 2048, 1024, 16
DK = D // H  # 64
NCORES = 8
HPC = H // NCORES  # 2 heads per core
KH = HPC * DK  # 128 = stacked head dim per core
NT = S // 128  # 16 t-tiles per batch
NI = S // 512  # 4 sq-blocks per batch
DC = D // 128  # 8 d-chunks

F32 = mybir.dt.float32
F32R = mybir.dt.float32r


def build_nc():
    nc = bacc.Bacc("TRN2", target_bir_lowering=False, debug=False,
                   num_devices=NCORES)
    x = nc.dram_tensor("x", [B, S, D], F32R, kind="ExternalInput").ap()
    wq = nc.dram_tensor("wq", [DC, 128, KH], F32R, kind="ExternalInput").ap()
    wk = nc.dram_tensor("wk", [DC, 128, KH], F32R, kind="ExternalInput").ap()
    wv = nc.dram_tensor("wv", [DC, 128, KH], F32R, kind="ExternalInput").ap()
    bq = nc.dram_tensor("bq", [KH, 1], F32, kind="ExternalInput").ap()
    bk = nc.dram_tensor("bk", [KH, 1], F32, kind="ExternalInput").ap()
    bv = nc.dram_tensor("bv", [KH, 1], F32, kind="ExternalInput").ap()
    wo = nc.dram_tensor("wo", [KH, D], F32R, kind="ExternalInput").ap()
    tri = nc.dram_tensor("tri", [128, 128], F32R, kind="ExternalInput").ap()
    out = nc.dram_tensor("out", [B, S, D], F32, kind="ExternalOutput").ap()

    with TileContext(nc) as tc:
        with (
            tc.tile_pool(name="const", bufs=1) as cpool,
            tc.tile_pool(name="wts", bufs=1) as wpool,
            tc.tile_pool(name="xt", bufs=2) as xpool,
            tc.tile_pool(name="seq", bufs=2) as qpool,
            tc.tile_pool(name="vn", bufs=2) as vpool,
            tc.tile_pool(name="attn", bufs=4) as apool,
            tc.tile_pool(name="fin", bufs=2) as fpool,
            tc.tile_pool(name="pacc", bufs=2, space="PSUM") as ps_acc,
            tc.tile_pool(name="psc", bufs=3, space="PSUM") as ps_sc,
            tc.tile_pool(name="pv", bufs=2, space="PSUM") as ps_v,
            tc.tile_pool(name="ptr", bufs=1, space="PSUM") as ps_tr,
        ):
            # persistent constants
            ident = cpool.tile([128, 128], F32, tag="ident")
            make_identity(nc, ident[:])
            ones_r = cpool.tile([128, 64], F32R, tag="ones")
            nc.gpsimd.memset(ones_r[:].bitcast(F32), 1.0)
            tri2_sb = cpool.tile([128, 256], F32R, tag="tri")
            nc.gpsimd.memset(tri2_sb[:, 0:128].bitcast(F32), 0.0)
            nc.sync.dma_start(out=tri2_sb[:, 128:256], in_=tri)
            w_sb = {}
            for nm, src in (("q", wq), ("k", wk), ("v", wv)):
                t = cpool.tile([128, DC * KH], F32R, tag="w" + nm)
                nc.sync.dma_start(
                    out=t[:].rearrange("p (c k) -> p c k", c=DC),
                    in_=src.rearrange("c p k -> p c k"))
                w_sb[nm] = t
            b_sb = {}
            for nm, src in (("q", bq), ("k", bk), ("v", bv)):
                t = cpool.tile([KH, 1], F32, tag="b" + nm)
                nc.sync.dma_start(out=t[:], in_=src)
                b_sb[nm] = t
            wo_sb = cpool.tile([KH, D], F32R, tag="wo")
            nc.sync.dma_start(out=wo_sb[:], in_=wo)

            for b in range(B):
                # ---------------- projections ----------------
                qt_sb = qpool.tile([128, S], F32R, tag="qt")
                kt_sb = qpool.tile([128, S], F32R, tag="kt")
                vn = [vpool.tile([128, 2 * DK + 2], F32R, tag=f"vn{j}",
                                 name=f"vn_{b}_{j}") for j in range(NT)]
                for st in range(NI):
                    xns = []
                    for ss in range(4):
                        xn = xpool.tile([128, D], F32, tag=f"xn{ss}",
                                        name=f"xn_{b}_{st}_{ss}")
                        nc.sync.dma_start(
                            out=xn[:],
                            in_=x[b, st * 512 + ss * 128:
                                  st * 512 + (ss + 1) * 128, :].bitcast(F32))
                        xns.append(xn)
                    xts = []
                    for dc in range(DC):
                        xt = xpool.tile([128, 512], F32R, tag=f"xt{dc}")
                        tps = ps_tr.tile([128, 512], F32, tag="tr",
                                         name=f"tps_{b}_{st}_{dc}")
                        for ss in range(4):
                            nc.tensor.transpose(
                                tps[:, ss * 128:(ss + 1) * 128],
                                xns[ss][:, dc * 128:(dc + 1) * 128],
                                ident[:])
                        nc.vector.tensor_copy(xt[:], tps[:])
                        xts.append(xt)
                    for nm, dst in (("q", qt_sb), ("k", kt_sb), ("v", None)):
                        acc = ps_acc.tile([128, 512], F32, tag="acc")
                        for dc in range(DC):
                            nc.tensor.matmul(
                                acc[:], w_sb[nm][:, dc * KH:(dc + 1) * KH],
                                xts[dc][:], start=(dc == 0), stop=(dc == DC - 1))
                        if nm != "v":
                            # psum -> sbuf with per-partition bias add
                            nc.vector.tensor_scalar_add(
                                dst[:, st * 512:(st + 1) * 512], acc[:],
                                b_sb[nm][:])
                        else:
                            vtt = fpool.tile([128, 512], F32, tag="vtt")
                            nc.vector.tensor_scalar_add(vtt[:], acc[:],
                                                        b_sb["v"][:])
                            for q in range(4):  # transpose to V natural
                                j = st * 4 + q
                                tp = ps_tr.tile([128, 128], F32, tag="tr")
                                nc.tensor.transpose(
                                    tp[:], vtt[:, q * 128:(q + 1) * 128],
                                    ident[:])
                                nc.vector.tensor_copy(vn[j][:, 0:DK],
                                                      tp[:, 0:DK])
                                nc.vector.tensor_copy(
                                    vn[j][:, DK + 1:2 * DK + 1],
                                    tp[:, DK:2 * DK])
                                nc.gpsimd.memset(vn[j][:, DK:DK + 1].bitcast(F32), 1.0)
                                nc.gpsimd.memset(
                                    vn[j][:, 2 * DK + 1:2 * DK + 2]
                                    .bitcast(F32), 1.0)

                # ---------------- attention ----------------
                catt = fpool.tile([128, S], F32R, tag="catt")
                for i in range(NI):
                    vps = [ps_v.tile([65, 512], F32, tag="vv",
                                     name=f"vp_{b}_{i}_{h}") for h in range(2)]
                    pend = None
                    for j in range(4 * i + 4):
                        doff = 128 * (j - 4 * i) if j >= 4 * i else 0
                        off = min(doff, 256)  # N=128 fp32r runs at 1/4 rate
                        w = 512 - off
                        sq0 = i * 512 + off
                        ats = []
                        for h in range(2):
                            kslc = slice(h * DK, (h + 1) * DK)
                            sp = ps_sc.tile([128, 512], F32, tag="sc",
                                            name=f"sp_{b}_{i}_{j}_{h}")
                            nc.tensor.matmul(
                                sp[:, off:], kt_sb[kslc, j * 128:(j + 1) * 128],
                                qt_sb[kslc, sq0:i * 512 + 512],
                                start=True, stop=True)
                            at = apool.tile([128, 512], F32R, tag=f"at{h}",
                                            name=f"at_{b}_{i}_{j}_{h}")
                            nc.scalar.activation(
                                at[:, 0:w], sp[:, off:],
                                mybir.ActivationFunctionType.Exp,
                                scale=float(1.0 / np.sqrt(DK)))
                            if j >= 4 * i:
                                mw = doff - off + 128
                                nc.vector.tensor_mul(
                                    at[:, 0:mw], at[:, 0:mw],
                                    tri2_sb[:, 256 - mw:256])
                            ats.append(at)
                        for h in range(2):
                            nc.tensor.matmul(
                                vps[h][:, off:],
                                vn[j][:, h * (DK + 1):(h + 1) * (DK + 1)],
                                ats[h][:, 0:w],
                                start=(j == 0), stop=(j == 4 * i + 3))
                    # normalize: recip of denom row, broadcast via PE, multiply
                    dn = fpool.tile([65, 1024], F32R, tag="dn")
                    for h in range(2):
                        nc.vector.tensor_copy(dn[64:65, h * 512:(h + 1) * 512],
                                              vps[h][64:65, :])
                    with nc.allow_low_precision(reason="f32r == f32 bytes"):
                        nc.vector.reciprocal(dn[64:65, :], dn[64:65, :])
                    nat1 = fpool.tile([64, 512], F32R, tag="nat1")
                    for h in range(2):
                        bc = ps_sc.tile([64, 512], F32, tag="sc")
                        nc.tensor.matmul(bc[:], ones_r[64:65, :],
                                         dn[64:65, h * 512:(h + 1) * 512],
                                         start=True, stop=True)
                        bcs = fpool.tile([64, 512], F32R, tag="bcs",
                                         name=f"bcs_{b}_{i}_{h}")
                        nc.vector.tensor_copy(bcs[:], bc[:])
                        dst = (catt[0:64, i * 512:(i + 1) * 512] if h == 0
                               else nat1[:])
                        nc.vector.tensor_mul(dst, vps[h][0:64, :], bcs[:])
                    # cross-partition hop: head1 rows into catt[64:128]
                    nc.sync.dma_start(
                        out=catt[64:128, i * 512:(i + 1) * 512], in_=nat1[:])

                # ---------------- output projection ----------------
                for st in range(NT):
                    ob = fpool.tile([128, D], F32, tag="ob")
                    for half in range(2):
                        pw = ps_acc.tile([128, 512], F32, tag="acc")
                        nc.tensor.matmul(
                            pw[:], catt[:, st * 128:(st + 1) * 128],
                            wo_sb[:, half * 512:(half + 1) * 512],
                            start=True, stop=True)
                        nc.vector.tensor_copy(
                            ob[:, half * 512:(half + 1) * 512], pw[:])
                    nc.sync.dma_start(
                        out=out[b, st * 128:(st + 1) * 128, :], in_=ob[:])
    nc.finalize()
    return nc


_NC_CACHE = {}


def _get_nc():
    if "nc" not in _NC_CACHE:
        _NC_CACHE["nc"] = build_nc()
    return _NC_CACHE["nc"]


def kernel(x, Wq, bq, Wk, bk, Wv, bv, Wo, bo):
    x = np.ascontiguousarray(np.asarray(x, dtype=np.float32))
    tri = np.triu(np.ones((128, 128), dtype=np.float32))
    in_maps = []
    for c in range(NCORES):
        h0, h1 = 2 * c, 2 * c + 1
        m = {
            "x": x,
            "tri": tri,
            "wo": np.ascontiguousarray(Wo[c * KH:(c + 1) * KH]).astype(
                np.float32),
        }
        for nm, W, bb in (("q", Wq, bq), ("k", Wk, bk), ("v", Wv, bv)):
            Wc = np.concatenate([W[h0], W[h1]], axis=1).astype(np.float32)
            m["w" + nm] = np.ascontiguousarray(Wc.reshape(DC, 128, KH))
            m["b" + nm] = np.concatenate([bb[h0], bb[h1]]).astype(
                np.float32).reshape(KH, 1)
        in_maps.append(m)
    nc = _get_nc()
    res = run_bass_kernel_spmd(nc, in_maps, list(range(NCORES)))
    acc = np.zeros((B, S, D), dtype=np.float32)
    for c in range(NCORES):
        acc += res.results[c]["out"]
    return acc + np.asarray(bo, dtype=np.float32)[None, None, :]
